# revision 3
# baseline (speedup 1.0000x reference)
"""MoE expert-parallel FFN kernel for TRN2 (8 NeuronCores).

Reference computation (per expert e):
    h = gelu(x_e @ W1[e] + b1[e]);  y_e = h @ W2[e] + b2[e]
with x = inputs[0].reshape(E, CAP, D), E=8, CAP=4096, D=1024, F=4096.

Sharding: expert parallel - core e owns expert e and its CAP-token slice.
No cross-core communication.

v2 design (all matmuls bf16, ~3e-3 rel err, well inside the 2e-2 gate):
  - W1 and W2 stay RESIDENT in SBUF (8MB + 8MB bf16); HBM traffic per core
    drops from ~160MB (v1 re-streamed weights per token tile) to ~40MB.
  - Token tiles of TC=512; x streamed (double-buffered), ht kept on-chip.
  - mm1: hT[f,tok] = W1.T @ xT, k-rotating stationary (8 distinct tiles);
    fused bias+gelu eviction on ACT.
  - mm2: y[tok,d] = hT.T @ W2, tm-rotating stationary (4 distinct tiles);
    b2 added by DVE at psum eviction.
  Consecutive matmuls never share a stationary tile: measured per-matmul
  cost is ~165ns (512-col stream) when rotating >=4 stationary tiles vs
  ~196ns (f32r) when repeating and ~755ns for bf16 with <4-tile rotation.
"""

import sys

if "/opt/trn_rl_repo" not in sys.path:
    sys.path.insert(0, "/opt/trn_rl_repo")

from contextlib import ExitStack

import numpy as np

import concourse.bacc as bacc
import concourse.tile as tile
from concourse import mybir
from concourse.bass_utils import run_bass_kernel_spmd

E, CAP, D, F = 8, 4096, 1024, 4096
P = 128
TC = 512             # tokens per tile
NT = CAP // TC       # token tiles per core (8)
KD = D // P          # k-tiles for mm1 (8)
FM = F // P          # f chunks (32)
TM = TC // P         # token sub-tiles for mm2 (4)
NDH = D // 512       # output d halves (2)

F32 = mybir.dt.float32
F32R = mybir.dt.float32r
BF16 = mybir.dt.bfloat16
GELU = mybir.ActivationFunctionType.Gelu_apprx_tanh

MM_DT = BF16

_cache = {}


def _build(mm_dt=None, repeat=1):
    if mm_dt is None:
        mm_dt = MM_DT
    nc = bacc.Bacc("TRN2", target_bir_lowering=False, debug=False)

    xt = nc.dram_tensor("xt", [D, CAP], mm_dt, kind="ExternalInput")
    w1t = nc.dram_tensor("w1t", [FM, P, KD, P], mm_dt, kind="ExternalInput")
    w2t = nc.dram_tensor("w2t", [FM, P, D], mm_dt, kind="ExternalInput")
    b1t = nc.dram_tensor("b1t", [P, FM], F32, kind="ExternalInput")
    b2b = nc.dram_tensor("b2b", [P, D], F32, kind="ExternalInput")
    y = nc.dram_tensor("y", [CAP, D], F32, kind="ExternalOutput")

    with tile.TileContext(nc) as tc:
        with ExitStack() as ctx:
            const = ctx.enter_context(tc.tile_pool(name="const", bufs=1))
            xpool = ctx.enter_context(tc.tile_pool(name="x", bufs=2))
            htpool = ctx.enter_context(tc.tile_pool(name="ht", bufs=1))
            ypool = ctx.enter_context(tc.tile_pool(name="yev", bufs=4))
            psum = ctx.enter_context(tc.tile_pool(name="psum", bufs=8, space="PSUM"))

            b1_sb = const.tile([P, FM], F32)
            nc.sync.dma_start(b1_sb[:], b1t.ap())
            b2_sb = const.tile([P, D], F32)
            nc.sync.dma_start(b2_sb[:], b2b.ap())

            xt_r = xt.ap().rearrange("(k p) c -> p k c", p=P)  # [128, KD, CAP]
            w1_r = w1t.ap()  # [FM, P, KD, P]
            w2_r = w2t.ap()  # [FM, P, D]
            y_r = y.ap()

            # Resident weights. Issue order: x tile 0 BEFORE the 16MB weight
            # preload (else the first matmul queues behind ~50us of weight
            # DMA), then w1 (mm1 of tile 0 consumes them progressively),
            # then w2 (not needed until ~60us in).
            w1_sb = [const.tile([P, KD, P], mm_dt, name=f"w1_{fm}") for fm in range(FM)]
            w2_sb = [const.tile([P, D], mm_dt, name=f"w2_{fm}") for fm in range(FM)]
            x_first = xpool.tile([P, KD, TC], mm_dt, tag="x")
            for k in range(KD):
                nc.sync.dma_start(x_first[:, k], xt_r[:, k, 0:TC])
            for fm in range(FM):
                nc.sync.dma_start(w1_sb[fm][:], w1_r[fm])
            for fm in range(FM):
                nc.sync.dma_start(w2_sb[fm][:], w2_r[fm])

            for t in [t for _ in range(repeat) for t in range(NT)]:
                if x_first is not None:
                    x_sb, x_first = x_first, None
                else:
                    x_sb = xpool.tile([P, KD, TC], mm_dt, tag="x")
                    for k in range(KD):
                        nc.sync.dma_start(
                            x_sb[:, k], xt_r[:, k, t * TC:(t + 1) * TC]
                        )

                ht_sb = htpool.tile([P, FM, TC], mm_dt, tag="ht")

                # --- mm1: hT[f_chunk, tok] += W1.T @ xT, fused bias+gelu ---
                for fm in range(FM):
                    ps = psum.tile([P, TC], F32, tag="ps", name="psh")
                    for k in range(KD):
                        nc.tensor.matmul(
                            ps[:],
                            w1_sb[fm][:, k],
                            x_sb[:, k],
                            start=(k == 0),
                            stop=(k == KD - 1),
                        )
                    nc.scalar.activation(
                        ht_sb[:, fm], ps[:], GELU, bias=b1_sb[:, fm:fm + 1]
                    )

                # --- mm2: y[tok, d] += hT.T @ W2, b2 added on eviction ---
                for dh in range(NDH):
                    ps_y = [
                        psum.tile([P, 512], F32, tag="ps", name="psy")
                        for _ in range(TM)
                    ]
                    for fm in range(FM):
                        for tm in range(TM):
                            nc.tensor.matmul(
                                ps_y[tm][:],
                                ht_sb[:, fm, tm * P:(tm + 1) * P],
                                w2_sb[fm][:, dh * 512:(dh + 1) * 512],
                                start=(fm == 0),
                                stop=(fm == FM - 1),
                            )
                    for tm in range(TM):
                        y_sb = ypool.tile([P, 512], F32, tag="y")
                        nc.vector.tensor_add(
                            y_sb[:], ps_y[tm][:], b2_sb[:, dh * 512:(dh + 1) * 512]
                        )
                        nc.sync.dma_start(
                            y_r[t * TC + tm * P:t * TC + (tm + 1) * P,
                                dh * 512:(dh + 1) * 512],
                            y_sb[:],
                        )

    nc.compile()
    return nc


def _wire_np_dtype(mm_dt):
    if mm_dt == BF16:
        import ml_dtypes

        return ml_dtypes.bfloat16
    return np.float32


def _prep_core_inputs(inputs, W1, b1, W2, b2, e, wdt):
    x_e = inputs[0, e * CAP:(e + 1) * CAP, :]          # [CAP, D]
    xt = np.ascontiguousarray(x_e.T).astype(wdt)       # [D, CAP]
    # kernel reads w1t[fm][p, k, f] == W1[k*P + p, fm*P + f]
    w1t = np.ascontiguousarray(
        W1[e].reshape(KD, P, FM, P).transpose(2, 1, 0, 3)
    ).astype(wdt)
    w2t = np.ascontiguousarray(W2[e].reshape(FM, P, D)).astype(wdt)
    b1t = np.ascontiguousarray(b1[e].reshape(FM, P).T)  # [P, FM]
    b2b = np.ascontiguousarray(np.broadcast_to(b2[e], (P, D)))
    return {
        "xt": xt,
        "w1t": w1t,
        "w2t": w2t,
        "b1t": b1t,
        "b2b": b2b,
    }


def get_nc(mm_dt=None, repeat=1, **_ignored):
    if mm_dt is None:
        mm_dt = MM_DT
    key = (mm_dt, repeat)
    if key not in _cache:
        _cache[key] = _build(mm_dt, repeat)
    return _cache[key]


def make_in_maps(inputs, W1, b1, W2, b2, mm_dt=None):
    inputs = np.asarray(inputs, dtype=np.float32)
    W1 = np.asarray(W1, dtype=np.float32)
    b1 = np.asarray(b1, dtype=np.float32)
    W2 = np.asarray(W2, dtype=np.float32)
    b2 = np.asarray(b2, dtype=np.float32)
    wdt = _wire_np_dtype(mm_dt if mm_dt is not None else MM_DT)
    return [_prep_core_inputs(inputs, W1, b1, W2, b2, e, wdt) for e in range(E)]


def kernel(inputs, W1, b1, W2, b2):
    nc = get_nc()
    in_maps = make_in_maps(inputs, W1, b1, W2, b2)
    # The axon-tunneled devices occasionally come up wedged from a previous
    # process (NRT_EXEC_UNIT_UNRECOVERABLE); a backend reset + retry recovers.
    last_err = None
    for attempt in range(3):
        try:
            res = run_bass_kernel_spmd(nc, in_maps, list(range(E))).results
            break
        except Exception as err:  # noqa: BLE001
            last_err = err
            import time as _time

            try:
                import jax as _jax
                import jax.extend.backend as _jxb

                _jax.clear_caches()
                _jxb.clear_backends()
            except Exception:  # noqa: BLE001
                pass
            _time.sleep(10.0 * (attempt + 1))
    else:
        raise last_err
    out = np.empty((1, E * CAP, D), dtype=np.float32)
    for e in range(E):
        out[0, e * CAP:(e + 1) * CAP, :] = res[e]["y"]
    return out


if __name__ == "__main__":
    rng = np.random.default_rng(0)
    ins = {
        "inputs": rng.standard_normal((1, E * CAP, D), dtype=np.float32),
        "W1": rng.standard_normal((E, D, F), dtype=np.float32) / np.sqrt(D),
        "b1": np.zeros((E, F), np.float32),
        "W2": rng.standard_normal((E, F, D), dtype=np.float32) / np.sqrt(F),
        "b2": np.zeros((E, D), np.float32),
    }
    y = kernel(**ins)
    print("out", y.shape, y.dtype, float(np.abs(y).mean()))


# revision 5
# speedup vs baseline: 1.0160x; 1.0160x over previous
"""MoE expert-parallel FFN kernel for TRN2 (8 NeuronCores).

Reference computation (per expert e):
    h = gelu(x_e @ W1[e] + b1[e]);  y_e = h @ W2[e] + b2[e]
with x = inputs[0].reshape(E, CAP, D), E=8, CAP=4096, D=1024, F=4096.

Sharding: expert parallel - core e owns expert e and its CAP-token slice.
No cross-core communication.

v3 design (all matmuls bf16, ~3e-3 rel err vs the 2e-2 gate):
  The PE-time floor is 4096 matmuls x 512 cols @2.4GHz = 874us/core; the
  measured hw overhead above that is ~53ns per matmul = exactly one
  serialized FWL bf16 LDWEIGHTS (128 cols / 2-per-cycle / 1.2GHz).  So v3
  is structured to AMORTIZE each weight load over 4 consecutive matmuls
  (same stationary, 4 x 512-token chunks into 4 psum banks), and walrus is
  run with --enable-ldw-opt=true to elide the redundant loads:
    mm1: token tiles of TC=2048; stationary w1[fm][k] streams 4 chunks.
    mm2: flipped to yT[d,tok] = W2.T @ hT -- stationary w2[fm][:,db*128:]
         streams 4 ht chunks; b2 added via ACT bias on eviction; the host
         transposes yT back.
  ldw-opt elision is broken for float32r (all-zero output / device wedge,
  see bass.ldweights docstring) but is exercised here only with bf16.
"""

import sys

if "/opt/trn_rl_repo" not in sys.path:
    sys.path.insert(0, "/opt/trn_rl_repo")

from contextlib import ExitStack

import numpy as np

import concourse.bacc as bacc
import concourse.tile as tile
from concourse import mybir
from concourse.bass_utils import run_bass_kernel_spmd

E, CAP, D, F = 8, 4096, 1024, 4096
P = 128
TC = 2048            # tokens per tile
NT = CAP // TC       # token tiles per core (2)
NC4 = TC // 512      # 512-token chunks per tile (4) = matmuls per weight load
KD = D // P          # k-tiles for mm1 (8)
FM = F // P          # f chunks (32)
DB = D // P          # output d blocks for mm2 (8)

F32 = mybir.dt.float32
F32R = mybir.dt.float32r
BF16 = mybir.dt.bfloat16
GELU = mybir.ActivationFunctionType.Gelu_apprx_tanh
IDENT = mybir.ActivationFunctionType.Identity

MM_DT = BF16
LDW_OPT = False  # walrus-side elision rejects pre-split LDW; we elide via
# InstMatmult.ldweights=False on trailing matmuls of same-stationary groups

_cache = {}


def _install_ldw_opt_patch():
    """Flip walrus's --enable-ldw-opt flag (redundant weight-load elision).

    Consecutive matmuls in this kernel share stationary operands; eliding
    the repeated LDWEIGHTS removes dead weight-load time on the PE.
    """
    import concourse.bass_utils as bu

    if getattr(bu, "_ldw_opt_patched", False):
        return
    orig = bu.run_command

    def patched(cmd, **kw):
        if LDW_OPT and isinstance(cmd, list):
            cmd = ["--enable-ldw-opt=true" if c == "--enable-ldw-opt=false" else c
                   for c in cmd]
        return orig(cmd, **kw)

    bu.run_command = patched
    bu._ldw_opt_patched = True


_install_ldw_opt_patch()


def _build(mm_dt=None, repeat=1):
    if mm_dt is None:
        mm_dt = MM_DT
    nc = bacc.Bacc("TRN2", target_bir_lowering=False, debug=False)

    xt = nc.dram_tensor("xt", [D, CAP], mm_dt, kind="ExternalInput")
    w1t = nc.dram_tensor("w1t", [FM, P, KD, P], mm_dt, kind="ExternalInput")
    # w2d[db, p, fm*128+d] == W2[fm*128+p, db*128+d]: one contiguous 1MB DMA
    # per output d-block, 8KB/partition lines
    w2d = nc.dram_tensor("w2d", [DB, P, FM * P], mm_dt, kind="ExternalInput")
    b1t = nc.dram_tensor("b1t", [P, FM], F32, kind="ExternalInput")
    b2t = nc.dram_tensor("b2t", [P, DB], F32, kind="ExternalInput")
    yt = nc.dram_tensor("yt", [D, CAP], F32, kind="ExternalOutput")

    with tile.TileContext(nc) as tc:
        with ExitStack() as ctx:
            const = ctx.enter_context(tc.tile_pool(name="const", bufs=1))
            xpool = ctx.enter_context(tc.tile_pool(name="x", bufs=1))
            htpool = ctx.enter_context(tc.tile_pool(name="ht", bufs=1))
            w1pool = ctx.enter_context(tc.tile_pool(name="w1", bufs=4))
            w2pool = ctx.enter_context(tc.tile_pool(name="w2", bufs=2))
            ypool = ctx.enter_context(tc.tile_pool(name="yev", bufs=4))
            psum = ctx.enter_context(tc.tile_pool(name="psum", bufs=8, space="PSUM"))

            b1_sb = const.tile([P, FM], F32)
            nc.sync.dma_start(b1_sb[:], b1t.ap())
            b2_sb = const.tile([P, DB], F32)
            nc.sync.dma_start(b2_sb[:], b2t.ap())

            xt_r = xt.ap().rearrange("(k p) c -> p k c", p=P)  # [128, KD, CAP]
            w1_r = w1t.ap()  # [FM, P, KD, P]
            w2_r = w2d.ap().rearrange("b p (f d) -> b p f d", f=FM)  # [DB,P,FM,128]
            yt_r = yt.ap()

            first = True
            for t in [t for _ in range(repeat) for t in range(NT)]:
                # x for this tile: issued before the first tile's weight
                # stream so the pipeline starts immediately
                x_sb = xpool.tile([P, KD, TC], mm_dt, tag="x")
                for k in range(KD):
                    nc.sync.dma_start(
                        x_sb[:, k], xt_r[:, k, t * TC:(t + 1) * TC]
                    )

                w1_next = w1pool.tile([P, KD, P], mm_dt, tag="w1")
                nc.sync.dma_start(w1_next[:], w1_r[0])
                if first:
                    w2_next = w2pool.tile([P, FM, P], mm_dt, tag="w2")
                    nc.sync.dma_start(w2_next[:], w2_r[0])
                    first = False

                ht_sb = htpool.tile([P, FM, TC], mm_dt, tag="ht")

                # --- mm1: hT[f,tok] = gelu(W1.T @ xT + b1) ---
                # k-rotating stationary; each w1[fm][k] streams NC4 chunks
                for fm in range(FM):
                    w1_sb = w1_next
                    if fm + 1 < FM:
                        w1_next = w1pool.tile([P, KD, P], mm_dt, tag="w1")
                        nc.sync.dma_start(w1_next[:], w1_r[fm + 1])
                    ps_h = [
                        psum.tile([P, 512], F32, tag="ps", name="psh")
                        for _ in range(NC4)
                    ]
                    for k in range(KD):
                        for c in range(NC4):
                            mm = nc.tensor.matmul(
                                ps_h[c][:],
                                w1_sb[:, k],
                                x_sb[:, k, c * 512:(c + 1) * 512],
                                start=(k == 0),
                                stop=(k == KD - 1),
                            )
                            if c > 0:
                                mm.ldweights = False
                    for c in range(NC4):
                        nc.scalar.activation(
                            ht_sb[:, fm, c * 512:(c + 1) * 512],
                            ps_h[c][:],
                            GELU,
                            bias=b1_sb[:, fm:fm + 1],
                        )

                # --- mm2: yT[d,tok] = W2.T @ hT + b2 (host transposes) ---
                # stationary w2[fm][:, db] streams NC4 ht chunks
                for db in range(DB):
                    w2_sb = w2_next
                    if db + 1 < DB:
                        w2_next = w2pool.tile([P, FM, P], mm_dt, tag="w2")
                        nc.sync.dma_start(w2_next[:], w2_r[db + 1])
                    else:
                        w2_next = w2pool.tile([P, FM, P], mm_dt, tag="w2")
                        nc.sync.dma_start(w2_next[:], w2_r[0])
                    ps_y = [
                        psum.tile([P, 512], F32, tag="ps", name="psy")
                        for _ in range(NC4)
                    ]
                    for fm in range(FM):
                        for c in range(NC4):
                            mm = nc.tensor.matmul(
                                ps_y[c][:],
                                w2_sb[:, fm],
                                ht_sb[:, fm, c * 512:(c + 1) * 512],
                                start=(fm == 0),
                                stop=(fm == FM - 1),
                            )
                            if c > 0:
                                mm.ldweights = False
                    for c in range(NC4):
                        y_sb = ypool.tile([P, 512], F32, tag="y")
                        nc.scalar.activation(
                            y_sb[:], ps_y[c][:], IDENT, bias=b2_sb[:, db:db + 1]
                        )
                        nc.sync.dma_start(
                            yt_r[db * P:(db + 1) * P,
                                 t * TC + c * 512:t * TC + (c + 1) * 512],
                            y_sb[:],
                        )

    nc.compile()
    return nc


def _wire_np_dtype(mm_dt):
    if mm_dt == BF16:
        import ml_dtypes

        return ml_dtypes.bfloat16
    return np.float32


def _prep_core_inputs(inputs, W1, b1, W2, b2, e, wdt):
    x_e = inputs[0, e * CAP:(e + 1) * CAP, :]          # [CAP, D]
    xt = np.ascontiguousarray(x_e.T).astype(wdt)       # [D, CAP]
    # kernel reads w1t[fm][p, k, f] == W1[k*P + p, fm*P + f]
    w1t = np.ascontiguousarray(
        W1[e].reshape(KD, P, FM, P).transpose(2, 1, 0, 3)
    ).astype(wdt)
    # w2d[db, p, fm*128+d] == W2[fm*128+p, db*128+d]
    w2d = np.ascontiguousarray(
        W2[e].reshape(FM, P, DB, P).transpose(2, 1, 0, 3).reshape(DB, P, FM * P)
    ).astype(wdt)
    b1t = np.ascontiguousarray(b1[e].reshape(FM, P).T)  # [P, FM]
    b2t = np.ascontiguousarray(b2[e].reshape(DB, P).T)  # [P, DB]
    return {
        "xt": xt,
        "w1t": w1t,
        "w2d": w2d,
        "b1t": b1t,
        "b2t": b2t,
    }


def get_nc(mm_dt=None, repeat=1, **_ignored):
    if mm_dt is None:
        mm_dt = MM_DT
    key = (mm_dt, repeat)
    if key not in _cache:
        _cache[key] = _build(mm_dt, repeat)
    return _cache[key]


def make_in_maps(inputs, W1, b1, W2, b2, mm_dt=None):
    inputs = np.asarray(inputs, dtype=np.float32)
    W1 = np.asarray(W1, dtype=np.float32)
    b1 = np.asarray(b1, dtype=np.float32)
    W2 = np.asarray(W2, dtype=np.float32)
    b2 = np.asarray(b2, dtype=np.float32)
    wdt = _wire_np_dtype(mm_dt if mm_dt is not None else MM_DT)
    return [_prep_core_inputs(inputs, W1, b1, W2, b2, e, wdt) for e in range(E)]


def output_from_results(res):
    out = np.empty((1, E * CAP, D), dtype=np.float32)
    for e in range(E):
        out[0, e * CAP:(e + 1) * CAP, :] = res[e]["yt"].T
    return out


def kernel(inputs, W1, b1, W2, b2):
    nc = get_nc()
    in_maps = make_in_maps(inputs, W1, b1, W2, b2)
    # The axon-tunneled devices occasionally come up wedged from a previous
    # process (NRT_EXEC_UNIT_UNRECOVERABLE); a backend reset + retry recovers.
    last_err = None
    for attempt in range(3):
        try:
            res = run_bass_kernel_spmd(nc, in_maps, list(range(E))).results
            break
        except Exception as err:  # noqa: BLE001
            last_err = err
            import time as _time

            try:
                import jax as _jax
                import jax.extend.backend as _jxb

                _jax.clear_caches()
                _jxb.clear_backends()
            except Exception:  # noqa: BLE001
                pass
            _time.sleep(10.0 * (attempt + 1))
    else:
        raise last_err
    return output_from_results(res)


if __name__ == "__main__":
    rng = np.random.default_rng(0)
    ins = {
        "inputs": rng.standard_normal((1, E * CAP, D), dtype=np.float32),
        "W1": rng.standard_normal((E, D, F), dtype=np.float32) / np.sqrt(D),
        "b1": np.zeros((E, F), np.float32),
        "W2": rng.standard_normal((E, F, D), dtype=np.float32) / np.sqrt(F),
        "b2": np.zeros((E, D), np.float32),
    }
    y = kernel(**ins)
    print("out", y.shape, y.dtype, float(np.abs(y).mean()))


# revision 7
# speedup vs baseline: 1.0238x; 1.0077x over previous
"""MoE expert-parallel FFN kernel for TRN2 (8 NeuronCores).

Reference computation (per expert e):
    h = gelu(x_e @ W1[e] + b1[e]);  y_e = h @ W2[e] + b2[e]
with x = inputs[0].reshape(E, CAP, D), E=8, CAP=4096, D=1024, F=4096.

Sharding: expert parallel - core e owns expert e and its CAP-token slice.
No cross-core communication.

v3 design (all matmuls bf16, ~3e-3 rel err vs the 2e-2 gate):
  The PE-time floor is 4096 matmuls x 512 cols @2.4GHz = 874us/core; the
  measured hw overhead above that is ~53ns per matmul = exactly one
  serialized FWL bf16 LDWEIGHTS (128 cols / 2-per-cycle / 1.2GHz).  So v3
  is structured to AMORTIZE each weight load over 4 consecutive matmuls
  (same stationary, 4 x 512-token chunks into 4 psum banks), and walrus is
  run with --enable-ldw-opt=true to elide the redundant loads:
    mm1: token tiles of TC=2048; stationary w1[fm][k] streams 4 chunks.
    mm2: flipped to yT[d,tok] = W2.T @ hT -- stationary w2[fm][:,db*128:]
         streams 4 ht chunks; b2 added via ACT bias on eviction; the host
         transposes yT back.
  ldw-opt elision is broken for float32r (all-zero output / device wedge,
  see bass.ldweights docstring) but is exercised here only with bf16.
"""

import sys

if "/opt/trn_rl_repo" not in sys.path:
    sys.path.insert(0, "/opt/trn_rl_repo")

from contextlib import ExitStack

import numpy as np

import concourse.bacc as bacc
import concourse.tile as tile
from concourse import mybir
from concourse.bass_utils import run_bass_kernel_spmd

E, CAP, D, F = 8, 4096, 1024, 4096
P = 128
TC = 2048            # tokens per tile
NT = CAP // TC       # token tiles per core (2)
NC4 = TC // 512      # 512-token chunks per tile (4) = matmuls per weight load
KD = D // P          # k-tiles for mm1 (8)
FM = F // P          # f chunks (32)
DB = D // P          # output d blocks for mm2 (8)

F32 = mybir.dt.float32
F32R = mybir.dt.float32r
BF16 = mybir.dt.bfloat16
GELU = mybir.ActivationFunctionType.Gelu_apprx_tanh
IDENT = mybir.ActivationFunctionType.Identity

MM_DT = BF16
LDW_OPT = False  # walrus-side elision rejects pre-split LDW; we elide via
# InstMatmult.ldweights=False on trailing matmuls of same-stationary groups

_cache = {}


def _install_ldw_opt_patch():
    """Flip walrus's --enable-ldw-opt flag (redundant weight-load elision).

    Consecutive matmuls in this kernel share stationary operands; eliding
    the repeated LDWEIGHTS removes dead weight-load time on the PE.
    """
    import concourse.bass_utils as bu

    if getattr(bu, "_ldw_opt_patched", False):
        return
    orig = bu.run_command

    def patched(cmd, **kw):
        if LDW_OPT and isinstance(cmd, list):
            cmd = ["--enable-ldw-opt=true" if c == "--enable-ldw-opt=false" else c
                   for c in cmd]
        return orig(cmd, **kw)

    bu.run_command = patched
    bu._ldw_opt_patched = True


_install_ldw_opt_patch()


def _dedup_ldweights(nc):
    """Drop redundant InstLdweights after bacc compile.

    bacc splits every InstMatmult into InstLdweights + InstMatmult
    (ldweights=False).  Within a same-stationary group (4 consecutive
    matmuls streaming different token chunks) the trailing loads are
    byte-identical reloads; deleting them removes ~53ns of serialized
    FWL weight-load per matmul on hw.  Only loads with NO semaphore
    waits/updates are deleted, so sync semantics are unchanged.
    """
    n_del = 0
    for fn in nc.m.functions:
        for bb in fn.blocks:
            cur = list(bb.instructions)
            out = []
            last_ldw_ap = None
            changed = False
            for ins in cur:
                nm = type(ins).__name__
                if nm == "InstLdweights":
                    ap = str(ins.ins[0])
                    if ap == last_ldw_ap and ins.sync_info is None:
                        n_del += 1
                        changed = True
                        continue
                    last_ldw_ap = ap
                elif nm == "InstMatmult":
                    if ins.ldweights is not False:
                        last_ldw_ap = None
                else:
                    try:
                        if ins.engine == mybir.EngineType.PE:
                            last_ldw_ap = None
                    except Exception:  # noqa: BLE001
                        last_ldw_ap = None
                out.append(ins)
            if changed:
                bb.instructions = out
    return n_del


def _build(mm_dt=None, repeat=1):
    if mm_dt is None:
        mm_dt = MM_DT
    nc = bacc.Bacc("TRN2", target_bir_lowering=False, debug=False)

    xt = nc.dram_tensor("xt", [D, CAP], mm_dt, kind="ExternalInput")
    w1t = nc.dram_tensor("w1t", [FM, P, KD, P], mm_dt, kind="ExternalInput")
    # w2d[db, p, fm*128+d] == W2[fm*128+p, db*128+d]: one contiguous 1MB DMA
    # per output d-block, 8KB/partition lines
    w2d = nc.dram_tensor("w2d", [DB, P, FM * P], mm_dt, kind="ExternalInput")
    b1t = nc.dram_tensor("b1t", [P, FM], F32, kind="ExternalInput")
    b2t = nc.dram_tensor("b2t", [P, DB], F32, kind="ExternalInput")
    yt = nc.dram_tensor("yt", [D, CAP], F32, kind="ExternalOutput")

    with tile.TileContext(nc) as tc:
        with ExitStack() as ctx:
            const = ctx.enter_context(tc.tile_pool(name="const", bufs=1))
            xpool = ctx.enter_context(tc.tile_pool(name="x", bufs=1))
            htpool = ctx.enter_context(tc.tile_pool(name="ht", bufs=1))
            w1pool = ctx.enter_context(tc.tile_pool(name="w1", bufs=4))
            w2pool = ctx.enter_context(tc.tile_pool(name="w2", bufs=2))
            ypool = ctx.enter_context(tc.tile_pool(name="yev", bufs=4))
            psum = ctx.enter_context(tc.tile_pool(name="psum", bufs=8, space="PSUM"))

            b1_sb = const.tile([P, FM], F32)
            nc.sync.dma_start(b1_sb[:], b1t.ap())
            b2_sb = const.tile([P, DB], F32)
            nc.sync.dma_start(b2_sb[:], b2t.ap())

            xt_r = xt.ap().rearrange("(k p) c -> p k c", p=P)  # [128, KD, CAP]
            w1_r = w1t.ap()  # [FM, P, KD, P]
            w2_r = w2d.ap().rearrange("b p (f d) -> b p f d", f=FM)  # [DB,P,FM,128]
            yt_r = yt.ap()

            first = True
            for t in [t for _ in range(repeat) for t in range(NT)]:
                # x for this tile: issued before the first tile's weight
                # stream so the pipeline starts immediately
                x_sb = xpool.tile([P, KD, TC], mm_dt, tag="x")
                for k in range(KD):
                    nc.sync.dma_start(
                        x_sb[:, k], xt_r[:, k, t * TC:(t + 1) * TC]
                    )

                w1_next = w1pool.tile([P, KD, P], mm_dt, tag="w1")
                nc.sync.dma_start(w1_next[:], w1_r[0])
                if first:
                    w2_next = w2pool.tile([P, FM, P], mm_dt, tag="w2")
                    nc.sync.dma_start(w2_next[:], w2_r[0])
                    first = False

                ht_sb = htpool.tile([P, FM, TC], mm_dt, tag="ht")

                # --- mm1: hT[f,tok] = gelu(W1.T @ xT + b1) ---
                # k-rotating stationary; each w1[fm][k] streams NC4 chunks
                for fm in range(FM):
                    w1_sb = w1_next
                    if fm + 1 < FM:
                        w1_next = w1pool.tile([P, KD, P], mm_dt, tag="w1")
                        nc.sync.dma_start(w1_next[:], w1_r[fm + 1])
                    ps_h = [
                        psum.tile([P, 512], F32, tag="ps", name="psh")
                        for _ in range(NC4)
                    ]
                    for k in range(KD):
                        for c in range(NC4):
                            mm = nc.tensor.matmul(
                                ps_h[c][:],
                                w1_sb[:, k],
                                x_sb[:, k, c * 512:(c + 1) * 512],
                                start=(k == 0),
                                stop=(k == KD - 1),
                            )
                            if c > 0:
                                mm.ldweights = False
                    for c in range(NC4):
                        nc.scalar.activation(
                            ht_sb[:, fm, c * 512:(c + 1) * 512],
                            ps_h[c][:],
                            GELU,
                            bias=b1_sb[:, fm:fm + 1],
                        )

                # --- mm2: yT[d,tok] = W2.T @ hT + b2 (host transposes) ---
                # stationary w2[fm][:, db] streams NC4 ht chunks
                for db in range(DB):
                    w2_sb = w2_next
                    if db + 1 < DB:
                        w2_next = w2pool.tile([P, FM, P], mm_dt, tag="w2")
                        nc.sync.dma_start(w2_next[:], w2_r[db + 1])
                    else:
                        w2_next = w2pool.tile([P, FM, P], mm_dt, tag="w2")
                        nc.sync.dma_start(w2_next[:], w2_r[0])
                    ps_y = [
                        psum.tile([P, 512], F32, tag="ps", name="psy")
                        for _ in range(NC4)
                    ]
                    for fm in range(FM):
                        for c in range(NC4):
                            mm = nc.tensor.matmul(
                                ps_y[c][:],
                                w2_sb[:, fm],
                                ht_sb[:, fm, c * 512:(c + 1) * 512],
                                start=(fm == 0),
                                stop=(fm == FM - 1),
                            )
                            if c > 0:
                                mm.ldweights = False
                    for c in range(NC4):
                        y_sb = ypool.tile([P, 512], F32, tag="y")
                        nc.scalar.activation(
                            y_sb[:], ps_y[c][:], IDENT, bias=b2_sb[:, db:db + 1]
                        )
                        nc.sync.dma_start(
                            yt_r[db * P:(db + 1) * P,
                                 t * TC + c * 512:t * TC + (c + 1) * 512],
                            y_sb[:],
                        )

    nc.compile()
    _dedup_ldweights(nc)
    return nc


def _wire_np_dtype(mm_dt):
    if mm_dt == BF16:
        import ml_dtypes

        return ml_dtypes.bfloat16
    return np.float32


def _prep_core_inputs(inputs, W1, b1, W2, b2, e, wdt):
    x_e = inputs[0, e * CAP:(e + 1) * CAP, :]          # [CAP, D]
    xt = np.ascontiguousarray(x_e.T).astype(wdt)       # [D, CAP]
    # kernel reads w1t[fm][p, k, f] == W1[k*P + p, fm*P + f]
    w1t = np.ascontiguousarray(
        W1[e].reshape(KD, P, FM, P).transpose(2, 1, 0, 3)
    ).astype(wdt)
    # w2d[db, p, fm*128+d] == W2[fm*128+p, db*128+d]
    w2d = np.ascontiguousarray(
        W2[e].reshape(FM, P, DB, P).transpose(2, 1, 0, 3).reshape(DB, P, FM * P)
    ).astype(wdt)
    b1t = np.ascontiguousarray(b1[e].reshape(FM, P).T)  # [P, FM]
    b2t = np.ascontiguousarray(b2[e].reshape(DB, P).T)  # [P, DB]
    return {
        "xt": xt,
        "w1t": w1t,
        "w2d": w2d,
        "b1t": b1t,
        "b2t": b2t,
    }


def get_nc(mm_dt=None, repeat=1, **_ignored):
    if mm_dt is None:
        mm_dt = MM_DT
    key = (mm_dt, repeat)
    if key not in _cache:
        _cache[key] = _build(mm_dt, repeat)
    return _cache[key]


def make_in_maps(inputs, W1, b1, W2, b2, mm_dt=None):
    inputs = np.asarray(inputs, dtype=np.float32)
    W1 = np.asarray(W1, dtype=np.float32)
    b1 = np.asarray(b1, dtype=np.float32)
    W2 = np.asarray(W2, dtype=np.float32)
    b2 = np.asarray(b2, dtype=np.float32)
    wdt = _wire_np_dtype(mm_dt if mm_dt is not None else MM_DT)
    return [_prep_core_inputs(inputs, W1, b1, W2, b2, e, wdt) for e in range(E)]


def output_from_results(res):
    out = np.empty((1, E * CAP, D), dtype=np.float32)
    for e in range(E):
        out[0, e * CAP:(e + 1) * CAP, :] = res[e]["yt"].T
    return out


def kernel(inputs, W1, b1, W2, b2):
    nc = get_nc()
    in_maps = make_in_maps(inputs, W1, b1, W2, b2)
    # The axon-tunneled devices occasionally come up wedged from a previous
    # process (NRT_EXEC_UNIT_UNRECOVERABLE); a backend reset + retry recovers.
    last_err = None
    for attempt in range(3):
        try:
            res = run_bass_kernel_spmd(nc, in_maps, list(range(E))).results
            break
        except Exception as err:  # noqa: BLE001
            last_err = err
            import time as _time

            try:
                import jax as _jax
                import jax.extend.backend as _jxb

                _jax.clear_caches()
                _jxb.clear_backends()
            except Exception:  # noqa: BLE001
                pass
            _time.sleep(10.0 * (attempt + 1))
    else:
        raise last_err
    return output_from_results(res)


if __name__ == "__main__":
    rng = np.random.default_rng(0)
    ins = {
        "inputs": rng.standard_normal((1, E * CAP, D), dtype=np.float32),
        "W1": rng.standard_normal((E, D, F), dtype=np.float32) / np.sqrt(D),
        "b1": np.zeros((E, F), np.float32),
        "W2": rng.standard_normal((E, F, D), dtype=np.float32) / np.sqrt(F),
        "b2": np.zeros((E, D), np.float32),
    }
    y = kernel(**ins)
    print("out", y.shape, y.dtype, float(np.abs(y).mean()))
